# revision 1
# baseline (speedup 1.0000x reference)
"""Trainium2 Bass kernel v8: cross-entropy with Gaussian-smoothed labels.

loss = mean over tokens of  [ Wsum(t) * logsumexp(pred_row) - sum_k w_k * pred[start+k] ]

Device-side, per core (8-way batch-parallel, 8192 tokens each):
  - stream pred [8192, 722] f32 through SBUF in [128, G*722] tiles;
    per-token sum(exp) is load-balanced between ACT (exp + HW
    accumulator, one op per token column) and DVE (batched exp +
    reduce_sum per group) so neither engine outruns the DMA stream;
    one Ln at the end -> lse.
  - the 7-tap windowed term: 64 indirect DMAs (one offset per
    partition per op - the only offset layout the SWDGE firmware
    honors; Q7 descriptor emission runs at ~8 ns/descriptor, so these
    ~71 us overlap the whole stream). Window offsets, tap weights and
    per-token weight sums are host-shipped (pure target preprocessing).
  - everything after the stream is wrapped in tile_wait_until so the
    Tile scheduler cannot hoist gather-consumers into the in-order DVE
    queue ahead of the stream reduces (its SWDGE cost model is ~10x
    optimistic, which otherwise head-of-line blocks the stream).
  - per-core partial sums [128, 1] DMA'd out; host sums 8x128 and divides.
"""
import math

import numpy as np

import concourse.bass as bass
import concourse.bacc as bacc
import concourse.tile as tile
from concourse import mybir
from concourse import bass_utils

B, T, C = 32, 2048, 722
CORES = 8
SHARD = B * T // CORES          # 8192 tokens per core
P = 128
TILES = SHARD // P              # 64
K = 7
START_MAX = C - K               # 715
DECAYS = [math.exp(-(2.0 ** d) / 4.0) for d in range(4)]

_ALU = mybir.AluOpType
_ACT = mybir.ActivationFunctionType

_F = np.zeros(13, np.float32)
for _m in range(4):
    _F[6 + _m] = _F[6 - _m] = DECAYS[_m]
_F[6] = 1.0    # exact target position overwrites decay[0]

_NC = None


def _build(G=4, acc_set=(3, 6, 9, 12, 15), pred_bufs=4, exp_bufs=3,
           wg_split=48, mid_wait_ms=0.08, tail_wait_ms=1.0):
    ngroups = TILES // G
    acc_set = set(acc_set)
    nc = bacc.Bacc("TRN2", target_bir_lowering=False, debug=False,
                   enable_asserts=True, num_devices=CORES,
                   dynamic_dma_scratch_size=32768)
    pred = nc.dram_tensor("pred", [SHARD, C], mybir.dt.float32, kind="ExternalInput")
    goffs = nc.dram_tensor("goffs", [SHARD], mybir.dt.int32, kind="ExternalInput")
    gwk = nc.dram_tensor("gwk", [SHARD * K], mybir.dt.float32, kind="ExternalInput")
    wsum_in = nc.dram_tensor("wsum", [SHARD], mybir.dt.float32, kind="ExternalInput")
    out = nc.dram_tensor("partial", [P, 1], mybir.dt.float32, kind="ExternalOutput")

    pred_flat = pred.ap().rearrange("a b -> (a b)").rearrange("(n one) -> n one", one=1)
    # token index = p*TILES + jg*G + g  (each partition owns a contiguous slab)
    pred_g = pred.ap().rearrange("(p j g) c -> j p g c", p=P, g=G)

    with tile.TileContext(nc) as tc:
        with (tc.tile_pool(name="pred", bufs=pred_bufs) as pred_pool,
              tc.tile_pool(name="exp", bufs=exp_bufs) as exp_pool,
              tc.tile_pool(name="small", bufs=1) as small):
            # host-precomputed gather constants via the scalar ring (sync
            # ring stays exclusive to the pred stream)
            # offs rides the sync ring FIRST so the gathers (the critical
            # path) start ~2us earlier; the pred stream queues behind it
            # with plenty of slack.
            offs = small.tile([P, TILES], mybir.dt.int32)
            nc.sync.dma_start(out=offs,
                              in_=goffs.ap().rearrange("(p j) -> p j", p=P))
            wk_sb = small.tile([P, TILES, K], mybir.dt.float32)
            nc.scalar.dma_start(
                out=wk_sb,
                in_=gwk.ap().rearrange("(p j k) -> p j k", p=P, k=K))
            wsum_sb = small.tile([P, TILES], mybir.dt.float32)
            nc.scalar.dma_start(out=wsum_sb,
                                in_=wsum_in.ap().rearrange("(p j) -> p j", p=P))

            # ordering fence: a Pool-engine op reading offs so the gathers
            # behind it on the Q7 queue cannot race the offs DMA
            offs_fence = small.tile([P, TILES], mybir.dt.int32)
            nc.gpsimd.tensor_copy(out=offs_fence, in_=offs)

            # windowed gathers: one indirect DMA per token-tile, one offset
            # per partition (the only HW-correct layout)
            gath = small.tile([P, TILES, K], mybir.dt.float32)
            for j in range(TILES):
                nc.gpsimd.indirect_dma_start(
                    out=gath[:, j, :],
                    out_offset=None,
                    in_=pred_flat,
                    in_offset=bass.IndirectOffsetOnAxis(ap=offs[:, j:j + 1], axis=0),
                )

            # dense stream: per-token sum(exp), split ACT-accum / DVE-reduce.
            # Every ACTIVATE carries accum_out (the HW accumulator register
            # is stateful across ops; pairing each op with a read is the
            # proven-correct pattern); batched groups dump theirs into a
            # junk column.
            sums = small.tile([P, TILES], mybir.dt.float32)
            junk = small.tile([P, 1], mybir.dt.float32)
            for jg in range(ngroups):
                pt = pred_pool.tile([P, G, C], mybir.dt.float32)
                nc.sync.dma_start(out=pt, in_=pred_g[jg])
                if jg in acc_set:
                    for g in range(G):
                        j = jg * G + g
                        et = exp_pool.tile([P, C], mybir.dt.float32, tag="acc")
                        nc.scalar.activation(out=et, in_=pt[:, g, :], func=_ACT.Exp,
                                             accum_out=sums[:, j:j + 1])
                else:
                    et = exp_pool.tile([P, G, C], mybir.dt.float32, tag="dve")
                    nc.scalar.activation(out=et, in_=pt, func=_ACT.Exp,
                                         accum_out=junk)
                    nc.vector.reduce_sum(out=sums[:, jg * G:(jg + 1) * G], in_=et,
                                         axis=mybir.AxisListType.X)

            # tail: force-scheduled after the stream (and after the gathers
            # finish for the DVE side) so nothing head-of-line blocks the
            # in-order engine queues mid-stream. The first wg_split token
            # columns' gathers are long done when the stream ends, so their
            # contraction runs in the post-stream window; only the last
            # columns gate on the final gathers.
            wg = small.tile([P, TILES, K], mybir.dt.float32)
            gsum = small.tile([P, TILES], mybir.dt.float32)
            loss = small.tile([P, TILES], mybir.dt.float32)
            lse = small.tile([P, TILES], mybir.dt.float32)
            with tc.tile_wait_until(mid_wait_ms):
                nc.scalar.activation(out=lse, in_=sums, func=_ACT.Ln)
                nc.vector.tensor_mul(out=loss, in0=wsum_sb, in1=lse)
                nc.vector.tensor_mul(out=wg[:, :wg_split, :],
                                     in0=wk_sb[:, :wg_split, :],
                                     in1=gath[:, :wg_split, :])
                nc.vector.reduce_sum(out=gsum[:, :wg_split],
                                     in_=wg[:, :wg_split, :],
                                     axis=mybir.AxisListType.X)
            with tc.tile_wait_until(tail_wait_ms):
                nc.vector.tensor_mul(out=wg[:, wg_split:, :],
                                     in0=wk_sb[:, wg_split:, :],
                                     in1=gath[:, wg_split:, :])
                nc.vector.reduce_sum(out=gsum[:, wg_split:],
                                     in_=wg[:, wg_split:, :],
                                     axis=mybir.AxisListType.X)
                nc.vector.tensor_sub(out=loss, in0=loss, in1=gsum)
                part = small.tile([P, 1], mybir.dt.float32)
                nc.vector.reduce_sum(out=part, in_=loss, axis=mybir.AxisListType.X)
                # scalar ring: the sync queue still has stream-completion
                # bookkeeping at this point, the ACT ring is idle
                nc.scalar.dma_start(out=out.ap(), in_=part)
    nc.compile()
    return nc


def _get_nc():
    global _NC
    if _NC is None:
        _NC = _build()
    return _NC


def _gather_plan(target_shard):
    """Host-side target preprocessing: flat window-start offsets, tap
    weights [SHARD, 7], per-token weight sums."""
    tgt = target_shard.astype(np.int64)
    s = np.clip(tgt - 3, 0, START_MAX)
    u = (tgt - s).astype(np.int64)
    t = np.arange(SHARD, dtype=np.int64)
    offs = (t * C + s).astype(np.int32)
    wk = _F[6 + (np.arange(K)[None, :] - u[:, None])].astype(np.float32)
    wsum = wk.sum(axis=1).astype(np.float32)
    # device layouts: token t at (p, j) = (t // TILES, t % TILES)
    tok = np.arange(SHARD).reshape(P, TILES)
    return (offs[tok].reshape(SHARD),
            wk[tok].reshape(SHARD * K),
            wsum[tok].reshape(SHARD))


def _shard_inputs(pred, target):
    bpc = B // CORES
    in_maps = []
    for c in range(CORES):
        tgt_shard = np.ascontiguousarray(
            target[c * bpc:(c + 1) * bpc].reshape(SHARD), dtype=np.int32)
        goffs, gwk, wsum = _gather_plan(tgt_shard)
        in_maps.append({
            "pred": np.ascontiguousarray(
                pred[c * bpc:(c + 1) * bpc].reshape(SHARD, C), dtype=np.float32),
            "goffs": goffs,
            "gwk": gwk,
            "wsum": wsum,
        })
    return in_maps


def _run(pred, target, **kwargs):
    nc = _get_nc()
    return bass_utils.run_bass_kernel_spmd(
        nc, _shard_inputs(pred, target), core_ids=list(range(CORES)), **kwargs)


def kernel(pred, target):
    res = _run(pred, target)
    total = sum(float(r["partial"].astype(np.float64).sum()) for r in res.results)
    return np.asarray(total / (B * T), dtype=np.float32)



# revision 2
# speedup vs baseline: 1.4405x; 1.4405x over previous
"""Trainium2 Bass kernel v9: cross-entropy with Gaussian-smoothed labels.

loss = mean over tokens of [ wsum(t) * logsumexp(pred_row) - sum_k w_k * pred[win_k] ]

Key ideas vs v8 (which spent ~90us on 64 indirect-DMA gathers):
  - The reference's scatter-with-clamp smoothed label reduces exactly to
    W[t,c] = g(|c - tgt_t|), g = [1, e^-.5, e^-1, e^-2], 0 beyond +-3
    (last-write-wins makes the smallest distance win at the boundaries).
  - The loss is a mean over tokens => permutation invariant. Host sorts
    tokens by target, so each group of 4 tiles (512 sorted tokens) has all
    its 7-wide windows inside one static 80-wide class band. The windowed
    term becomes 16 rectangular bf16 multiplies against host-shipped exact
    band weights; zero indirect DMAs.
  - rel-err gate is 2e-2: stream pred in bf16 (11.8 MB/core instead of
    23.7), measured end-to-end error ~1e-5.
  - per-token sum(exp) split between ACT (exp + HW accumulator per token
    column) and DVE (batched exp + reduce) so both engines finish together
    (~46us each); DVE TensorReduce has no 2x bf16 mode so ACT takes ~20
    columns.
  - per-core partial sums [128,1] f32 DMA'd out; host sums in f64.
"""
import math

import numpy as np
import ml_dtypes

import concourse.bass as bass
import concourse.bacc as bacc
import concourse.tile as tile
from concourse import mybir
from concourse import bass_utils

B, T, C = 32, 2048, 722
CORES = 8
SHARD = B * T // CORES          # 8192 tokens per core
P = 128
TILES = SHARD // P              # 64 token tiles of 128
G = 8                           # tiles per stream group (one DMA each)
NG = TILES // G                 # 8 stream groups
GB = 4                          # tiles per band group (shared class band)
NGB = TILES // GB               # 16 band groups
WIDTH = 80                      # static band width (seed-0 data needs 65)
DECAYS = [math.exp(-(2.0 ** d) / 4.0) for d in range(4)]
# accum columns per stream group (ACT-side sum(exp)); rest go DVE reduce
NA = (3, 2, 3, 2, 3, 2, 3, 2)

BF16 = ml_dtypes.bfloat16

_ALU = mybir.AluOpType
_ACT = mybir.ActivationFunctionType

# g(d) lookup, exact reference decay values (g(0)=1 from the final set())
_GVAL = np.zeros(8, np.float32)
for _d in range(4):
    _GVAL[_d] = 1.0 if _d == 0 else DECAYS[_d]

_NC_CACHE = {}


def _build(band_starts, pred_bufs=4, exp_bufs=3):
    nc = bacc.Bacc("TRN2", target_bir_lowering=False, debug=False,
                   enable_asserts=True, num_devices=CORES)
    pred = nc.dram_tensor("pred", [SHARD, C], mybir.dt.bfloat16,
                          kind="ExternalInput")
    wband = nc.dram_tensor("wband", [P, TILES * WIDTH], mybir.dt.bfloat16,
                           kind="ExternalInput")
    wsum_in = nc.dram_tensor("wsum", [P, TILES], mybir.dt.float32,
                             kind="ExternalInput")
    out = nc.dram_tensor("partial", [P, 1], mybir.dt.float32,
                         kind="ExternalOutput")

    # HBM row r = jg*(P*G) + p*G + g holds token sorted[(jg*G+g)*P + p]:
    # per-partition contiguous G rows -> one clean 11.5KB descriptor set.
    pred_r = pred.ap().rearrange("(j p g) c -> j p g c", p=P, g=G)

    with tile.TileContext(nc) as tc:
        with (tc.tile_pool(name="pred", bufs=pred_bufs) as pred_pool,
              tc.tile_pool(name="exp", bufs=exp_bufs) as exp_pool,
              tc.tile_pool(name="small", bufs=1) as small):
            # host-shipped band weights + weight sums ride the scalar ring
            # (sync ring stays exclusive to the pred stream)
            W_sb = small.tile([P, TILES, WIDTH], mybir.dt.bfloat16)
            nc.scalar.dma_start(
                out=W_sb,
                in_=wband.ap().rearrange("p (j w) -> p j w", w=WIDTH))
            wsum_sb = small.tile([P, TILES], mybir.dt.float32)
            nc.scalar.dma_start(out=wsum_sb, in_=wsum_in.ap())

            sums = small.tile([P, TILES], mybir.dt.float32)
            m = small.tile([P, TILES, WIDTH], mybir.dt.bfloat16)
            gsum = small.tile([P, TILES], mybir.dt.float32)
            junk = small.tile([P, C], mybir.dt.bfloat16)
            junk1 = small.tile([P, 1], mybir.dt.float32)

            for jg in range(NG):
                pt = pred_pool.tile([P, G, C], mybir.dt.bfloat16)
                nc.sync.dma_start(out=pt, in_=pred_r[jg])
                na = NA[jg]
                # ACT-side token sums: exp with HW accumulator, one op per
                # token column; exp values land in a junk tile.
                for g in range(na):
                    j = jg * G + g
                    nc.scalar.activation(out=junk, in_=pt[:, g, :],
                                         func=_ACT.Exp,
                                         accum_out=sums[:, j:j + 1])
                # DVE-side: one batched exp (accumulator paired with a junk
                # read, same proven pattern as v8), then a bf16 reduce.
                et = exp_pool.tile([P, G - 2, C], mybir.dt.bfloat16)
                nb = G - na
                nc.scalar.activation(out=et[:, :nb, :], in_=pt[:, na:, :],
                                     func=_ACT.Exp, accum_out=junk1)
                nc.vector.reduce_sum(out=sums[:, jg * G + na:(jg + 1) * G],
                                     in_=et[:, :nb, :],
                                     axis=mybir.AxisListType.X)
                # windowed term: band slices x host weights (bf16, 2x DVE)
                for b in range(G // GB):
                    gi = jg * (G // GB) + b
                    j0 = jg * G + b * GB
                    s = band_starts[gi]
                    nc.vector.tensor_mul(
                        out=m[:, j0:j0 + GB, :],
                        in0=pt[:, b * GB:(b + 1) * GB, s:s + WIDTH],
                        in1=W_sb[:, j0:j0 + GB, :])
                nc.vector.reduce_sum(out=gsum[:, jg * G:(jg + 1) * G],
                                     in_=m[:, jg * G:(jg + 1) * G, :],
                                     axis=mybir.AxisListType.X)

            # tail: ~1.5us
            lse = small.tile([P, TILES], mybir.dt.float32)
            loss = small.tile([P, TILES], mybir.dt.float32)
            part = small.tile([P, 1], mybir.dt.float32)
            nc.scalar.activation(out=lse, in_=sums, func=_ACT.Ln)
            nc.vector.tensor_mul(out=loss, in0=wsum_sb, in1=lse)
            nc.vector.tensor_sub(out=loss, in0=loss, in1=gsum)
            nc.vector.reduce_sum(out=part, in_=loss,
                                 axis=mybir.AxisListType.X)
            nc.scalar.dma_start(out=out.ap(), in_=part)
    nc.compile()
    return nc


def _get_nc(band_starts):
    key = tuple(band_starts)
    if key not in _NC_CACHE:
        _NC_CACHE[key] = _build(key)
    return _NC_CACHE[key]


def _band_starts(target):
    """Static per-band-group class-band starts, shared by all cores."""
    lo = np.full(NGB, 1 << 30, np.int64)
    hi = np.full(NGB, -1, np.int64)
    bpc = B // CORES
    for c in range(CORES):
        tg = np.sort(target[c * bpc:(c + 1) * bpc].reshape(-1))
        blocks = tg.reshape(NGB, GB * P)
        lo = np.minimum(lo, np.clip(blocks.min(axis=1) - 3, 0, C - 1))
        hi = np.maximum(hi, np.clip(blocks.max(axis=1) + 3, 0, C - 1))
    assert (hi - lo + 1).max() <= WIDTH, "band width exceeded"
    s = np.clip((lo + hi + 1 - WIDTH) // 2, 0, C - WIDTH)
    assert np.all((lo >= s) & (hi < s + WIDTH))
    return tuple(int(x) for x in s)


def _shard_inputs(pred, target, band_starts):
    bpc = B // CORES
    s_per_tile = np.asarray(band_starts, np.int64)[np.arange(TILES) // GB]
    i_idx = np.arange(WIDTH)
    in_maps = []
    for c in range(CORES):
        shard_pred = pred[c * bpc:(c + 1) * bpc].reshape(SHARD, C)
        tg = np.ascontiguousarray(
            target[c * bpc:(c + 1) * bpc].reshape(SHARD)).astype(np.int64)
        order = np.argsort(tg, kind="stable")
        # HBM row r = jg*(P*G) + p*G + g  <->  token order[(jg*G+g)*P + p]
        r = np.arange(SHARD)
        jgr, rem = np.divmod(r, P * G)
        pr, gr = np.divmod(rem, G)
        perm = order[(jgr * G + gr) * P + pr]
        pred_rows = np.ascontiguousarray(shard_pred[perm]).astype(BF16)
        # W[p, j, i] = g(|band_col - tgt|), exact reference weights
        tgt_pj = tg[order].reshape(TILES, P).T              # (P, TILES)
        cpos = s_per_tile[None, :, None] + i_idx[None, None, :]
        d = np.abs(cpos - tgt_pj[:, :, None])               # (P, TILES, W)
        W = _GVAL[np.minimum(d, 7)]                         # f32 exact
        wsum = W.sum(axis=2, dtype=np.float32)              # (P, TILES) f32
        in_maps.append({
            "pred": pred_rows,
            "wband": np.ascontiguousarray(W.astype(BF16).reshape(P, -1)),
            "wsum": np.ascontiguousarray(wsum),
        })
    return in_maps


def _run(pred, target, **kwargs):
    pred = np.asarray(pred)
    target = np.asarray(target)
    band_starts = _band_starts(target)
    nc = _get_nc(band_starts)
    return bass_utils.run_bass_kernel_spmd(
        nc, _shard_inputs(pred, target, band_starts),
        core_ids=list(range(CORES)), **kwargs)


def kernel(pred, target):
    res = _run(pred, target)
    total = sum(float(r["partial"].astype(np.float64).sum())
                for r in res.results)
    return np.asarray(total / (B * T), dtype=np.float32)
